# revision 20
# baseline (speedup 1.0000x reference)
"""Multi-head attention (B=1, S=4096, D=768, H=12) on 8 trn2 NeuronCores — v2.4.

Sharding: data-parallel over query rows (512 rows/core).
Per core:
  - Q/K/V projected with fp16 matmuls (transposed layouts; K/Q chunk-major)
  - attention runs over HEAD PAIRS (2h, 2h+1): their K/Q rows live at
    partitions 0-63 / 64-127 of the same dt block, so the two heads' score
    matmuls land in disjoint PE row-groups and execute CONCURRENTLY
    (row-tiling; tile_position auto-derived from base_partition —
    HW-verified ~2x)
  - per 3-ktile group: one exp ACT instruction per head over a [128,3,512]
    PSUM tile (amortizes ACT instruction overhead), mask applied
    multiplicatively on DVE (fp16 2x mode), PV accumulated into per-head
    ctx PSUM banks with an appended ones-column producing the softmax
    denominator for free
  - software-pipelined emission: scores(g+1) before pv(g-LAG), LAG=1,
    probs pool 3-deep so the ACT stream never stalls on slot reuse
  - V projection computes only this core's 512-row shard FIRST, then an
    8-core AllGather assembles the full V while the K projection runs (the
    redundant per-core V projection was 54us of serial PE time; the gather
    hides entirely under K proj)
  - K/Q/O projection bias-adds ride the otherwise-idle ACT engine
    (Identity with per-partition AP bias) instead of DVE, whose per-op
    pipe-flush DRAIN was throttling the projection phase
  - PSUM: sce 3 + sco 3 + ctxe 1 + ctxo 1 = 8 banks
"""

import ml_dtypes
import numpy as np

import concourse.bass as bass
import concourse.mybir as mybir
import concourse.tile as tile
from concourse import bacc, bass_utils

B, S, D, H = 1, 4096, 768, 12
DK = D // H  # 64
NCORES = 8
SQ = S // NCORES  # 512 query rows per core
KT_TILES = S // 128  # 32 k tiles
DT = D // 128  # 6 tiles of the model dim
NCH = S // 512  # 8 column chunks for full-seq projections
NPAIR = H // 2  # 6 head pairs

F16 = mybir.dt.float16
F32 = mybir.dt.float32
F8 = mybir.dt.float8e4

# ktile groups per head pair: 3 ktiles x 2 heads = 6 PSUM banks per group
GROUPS = [3] * 10 + [2]
assert sum(GROUPS) == KT_TILES

LAG = 1      # pv trails the exp pipeline by one group
SC_INTER = True  # (even, odd) score matmuls adjacent -> PE row-group pairing

_CACHE = {}


def build_kernel(variant="full", timing=False, sfx="", repeat=1):
    nc = bacc.Bacc("TRN2", target_bir_lowering=False, debug=False, num_devices=NCORES)

    kw = {} if timing else {"kind": "ExternalInput"}
    qT = nc.dram_tensor(f"qT{sfx}", [128, DT, SQ], F16, **kw)
    kT = nc.dram_tensor(f"kT{sfx}", [128, NCH, DT, 512], F16, **kw)
    vT = nc.dram_tensor(f"vT{sfx}", [128, DT, 512], F16, **kw)  # per-core v shard
    vsh = nc.dram_tensor(f"vsh{sfx}", [128, 4, H, DK + 1], F16, kind="Internal")
    vfull = nc.dram_tensor(
        f"vfull{sfx}", [NCORES * 128, 4, H, DK + 1], F16,
        kind="Internal", addr_space="Shared",
    )
    maskT = nc.dram_tensor(f"maskT{sfx}", [128, KT_TILES, SQ // 4], F8, **kw)
    w = {x: nc.dram_tensor(f"w{x}{sfx}", [128, DT, D], F16, **kw) for x in "qkvo"}
    bcat = nc.dram_tensor(f"bcat{sfx}", [128, 4 * DT + H * DK], F32, **kw)
    if timing:
        nc.dram_tensor(f"tinput{sfx}", [1, 8], F32, kind="ExternalInput")
    outT = nc.dram_tensor(f"outT{sfx}", [D, SQ], F16, kind="ExternalOutput")

    with tile.TileContext(nc) as tc:
        for _ in range(repeat):
            _build_tile(tc, qT, kT, vT, vsh, vfull, maskT, w, bcat, outT, variant)
    nc.compile()
    return nc


def _build_tile(tc, qT, kT, vT, vsh, vfull, maskT, w, bcat, outT, variant="full"):
    nc = tc.nc
    do_proj = variant != "attnonly"
    do_attn = variant != "proj"

    with (
        tc.tile_pool(name="persist", bufs=1) as persist,
        tc.tile_pool(name="stage", bufs=2) as stage,
        tc.tile_pool(name="wpool", bufs=2) as wpool,
        tc.tile_pool(name="probs", bufs=3) as probs_pool,
        tc.tile_pool(name="small", bufs=1) as small,
    ):
        # ---- persistent SBUF tensors ----
        maskT_sb = persist.tile([128, KT_TILES, SQ], F16)
        KT_sb = persist.tile([128, DT, S], F16)
        # mask ships with FOUR {0,1} bits packed per fp8 value (exact in
        # fp8): quarter the HBM bytes. The casting DMA widens to fp16 into a
        # scratch region borrowed from KT_sb's d=5 row (K proj writes it
        # last; sub-tile dep tracking serializes the few overlapping
        # bias-adds), then DVE unpacks arithmetically.
        maskP_sb = KT_sb[:, DT - 1, :].rearrange("p (k w) -> p k w", w=SQ // 4)
        nc.gpsimd.dma_start(out=maskP_sb, in_=maskT[:])
        mview = maskT_sb[:].rearrange("p k (w b) -> p k w b", b=4)
        for bi in (3, 2, 1, 0):
            nc.vector.tensor_scalar(
                out=mview[:, :, :, bi], in0=maskP_sb,
                scalar1=float(2 ** bi), scalar2=None,
                op0=mybir.AluOpType.is_ge,
            )
            if bi > 0:
                nc.vector.scalar_tensor_tensor(
                    out=maskP_sb, in0=mview[:, :, :, bi],
                    scalar=float(-(2 ** bi)), in1=maskP_sb,
                    op0=mybir.AluOpType.mult, op1=mybir.AluOpType.add,
                )

        V_sb = persist.tile([128, KT_TILES, H, DK + 1], F16)
        QT_sb = persist.tile([128, DT, SQ], F16)
        ctx_sb = persist.tile([128, DT, SQ], F16)

        bcat_sb = persist.tile([128, 4 * DT + H * DK], F32)
        nc.sync.dma_start(out=bcat_sb[:], in_=bcat[:])
        bias_sb = {x: bcat_sb[:, i * DT : (i + 1) * DT] for i, x in enumerate("qkvo")}
        # bv replicated across partitions (host-prepared) for the V projection
        bv_rep = bcat_sb[:, 4 * DT :].rearrange("p (h e) -> p h e", e=DK)

        ones_col = persist.tile([128, H, 1], F16)
        nc.vector.memset(ones_col[:], 1.0)

        # preload the exp ACT table set while the projections run
        warm = small.tile([128, 8], F16, tag="warm")
        nc.scalar.activation(out=warm[:], in_=ones_col[:, 0:8, 0],
                             func=mybir.ActivationFunctionType.Exp,
                             bias=0.0, scale=1.0)

        if not do_proj:
            nc.vector.memset(KT_sb[:, :, 0:512], 0.25)
            nc.vector.memset(V_sb[:], 0.25)
            nc.vector.memset(QT_sb[:], 0.25)
        if not do_attn:
            nc.vector.tensor_copy(out=ctx_sb[:], in_=QT_sb[:])

        if do_proj:
          with tc.tile_pool(name="pproj", bufs=4, space="PSUM") as pproj:
            # ---- V projection: own 512-row shard only, then AllGather ----
            # (each core receives a DIFFERENT vT input: its shard of v)
            wv_sb = wpool.tile([128, DT, D], F16, tag="w")
            nc.sync.dma_start(out=wv_sb[:], in_=w["v"][:])
            xv_sb = stage.tile([128, DT, 512], F16, tag="xT")
            nc.sync.dma_start(out=xv_sb[:], in_=vT[:])
            vsh_sb = stage.tile([128, 4, H, DK + 1], F16, tag="vsh", bufs=1)
            for rt in range(4):
                for half in range(2):
                    ps = pproj.tile([128, 384], F32, tag="pv", bufs=2)
                    for ka in range(DT):
                        nc.tensor.matmul(
                            ps[:],
                            xv_sb[:, ka, rt * 128 : (rt + 1) * 128],
                            wv_sb[:, ka, half * 384 : (half + 1) * 384],
                            start=(ka == 0),
                            stop=(ka == DT - 1),
                        )
                    nc.vector.tensor_add(
                        out=vsh_sb[:, rt, half * 6 : (half + 1) * 6, 0:DK],
                        in0=ps[:].rearrange("p (h e) -> p h e", e=DK),
                        in1=bv_rep[:, half * 6 : (half + 1) * 6, :],
                    )
                nc.vector.tensor_copy(
                    out=vsh_sb[:, rt, :, DK : DK + 1], in_=ones_col[:]
                )
            nc.sync.dma_start(out=vsh[:], in_=vsh_sb[:])
            nc.gpsimd.collective_compute(
                "AllGather",
                mybir.AluOpType.bypass,
                ins=[vsh[:]],
                outs=[vfull[:]],
                replica_groups=[list(range(NCORES))],
            )
            # regather: vfull[(c p) rt h e] -> V_sb[p (c rt) h e]
            nc.sync.dma_start(
                out=V_sb[:].rearrange("p (c rt) h e -> p c rt h e", c=NCORES),
                in_=vfull[:].rearrange("(c p) rt h e -> p c rt h e", c=NCORES),
            )

            # ---- K projection -> KT_sb (transposed layout) ----
            wk_sb = wpool.tile([128, DT, D], F16, tag="w")
            nc.sync.dma_start(out=wk_sb[:], in_=w["k"][:])
            for nch in range(NCH):
                x_sb = stage.tile([128, DT, 512], F16, tag="xT")
                nc.sync.dma_start(out=x_sb[:], in_=kT[:, nch])
                for d in range(DT):
                    ps = pproj.tile([128, 512], F32, tag="pj", bufs=6)
                    for ka in range(DT):
                        nc.tensor.matmul(
                            ps[:],
                            wk_sb[:, ka, d * 128 : (d + 1) * 128],
                            x_sb[:, ka, :],
                            start=(ka == 0),
                            stop=(ka == DT - 1),
                        )
                    nc.scalar.activation(
                        out=KT_sb[:, d, nch * 512 : (nch + 1) * 512],
                        in_=ps[:],
                        func=mybir.ActivationFunctionType.Identity,
                        bias=bias_sb["k"][:, d : d + 1],
                        scale=1.0,
                    )

            # ---- Q projection -> QT_sb ----
            wq_sb = wpool.tile([128, DT, D], F16, tag="w")
            nc.sync.dma_start(out=wq_sb[:], in_=w["q"][:])
            xq_sb = stage.tile([128, DT, 512], F16, tag="xT")
            nc.sync.dma_start(out=xq_sb[:], in_=qT[:])
            for d in range(DT):
                ps = pproj.tile([128, 512], F32, tag="pj", bufs=6)
                for ka in range(DT):
                    nc.tensor.matmul(
                        ps[:],
                        wq_sb[:, ka, d * 128 : (d + 1) * 128],
                        xq_sb[:, ka, :],
                        start=(ka == 0),
                        stop=(ka == DT - 1),
                    )
                nc.scalar.activation(
                    out=QT_sb[:, d, :],
                    in_=ps[:],
                    func=mybir.ActivationFunctionType.Identity,
                    bias=bias_sb["q"][:, d : d + 1],
                    scale=1.0,
                )

        # ---- attention: head pairs, pipelined groups ----
        if do_attn:
          with tc.tile_pool(name="pattn", bufs=1, space="PSUM") as pattn:
            sched = []
            for p in range(NPAIR):
                off = 0
                for g in GROUPS:
                    sched.append((p, off, g))
                    off += g
            NG = len(sched)

            sc_tiles = {}
            pr_tiles = {}
            ctx_pair = {}

            def emit_front(gi):
                p, off, g = sched[gi]
                sc_e = pattn.tile([128, 3, 512], F32, tag="sce")
                sc_o = pattn.tile([128, 3, 512], F32, tag="sco")
                # (even, odd) adjacent per ktile: disjoint PE row-groups run
                # concurrently (row-tiling)
                order = (
                    [(j, e) for j in range(g) for e in range(2)]
                    if SC_INTER
                    else [(j, e) for e in range(2) for j in range(g)]
                )
                for j, e in order:
                    kt = off + j
                    sc = (sc_e, sc_o)[e]
                    po = 64 * e
                    nc.tensor.matmul(
                        sc[:, j, :],
                        KT_sb[po : po + 64, p, kt * 128 : (kt + 1) * 128],
                        QT_sb[po : po + 64, p, :],
                        start=True,
                        stop=True,
                    )
                sc_tiles[gi] = (sc_e, sc_o)

            def emit_mid(gi):
                p, off, g = sched[gi]
                pr = probs_pool.tile([128, 2, 3, 512], F16, tag="probs")
                for e in range(2):
                    # ACT reads score PSUM directly (1/sqrt(dk) folded into
                    # the free affine scale), writes raw exp to SBUF fp16;
                    # DVE applies the {0,1} mask multiplicatively at 2x rate
                    nc.scalar.activation(
                        out=pr[:, e, 0:g, :],
                        in_=sc_tiles[gi][e][:, 0:g, :],
                        func=mybir.ActivationFunctionType.Exp,
                        bias=0.0,
                        scale=float(1.0 / np.sqrt(DK)),
                    )
                    nc.vector.tensor_mul(
                        out=pr[:, e, 0:g, :],
                        in0=pr[:, e, 0:g, :],
                        in1=maskT_sb[:, off : off + g, :],
                    )
                pr_tiles[gi] = pr
                del sc_tiles[gi]

            def emit_pv(gi):
                p, off, g = sched[gi]
                if off == 0:
                    ctx_e = pattn.tile([128, 512], F32, tag="ctxe")
                    ctx_o = pattn.tile([128, 512], F32, tag="ctxo")
                    ctx_pair[p] = (ctx_e, ctx_o)
                pr = pr_tiles.pop(gi)
                for j in range(g):
                    kt = off + j
                    for e in range(2):
                        h = 2 * p + e
                        nc.tensor.matmul(
                            ctx_pair[p][e][0 : DK + 1, :],
                            V_sb[:, kt, h, :],
                            pr[:, e, j, :],
                            start=(kt == 0),
                            stop=(kt == KT_TILES - 1),
                            skip_group_check=True,
                        )
                if off + g == KT_TILES:
                    # normalize: rows 0..63 are ctx^T, row 64 the denominator
                    for e in range(2):
                        po = 64 * e
                        cps = ctx_pair[p][e]
                        recip = small.tile([1, SQ], F32, tag="recip")
                        nc.vector.reciprocal(out=recip[:], in_=cps[DK : DK + 1, :])
                        recip_rep = small.tile([DK, SQ], F32, tag="recip_rep")
                        nc.gpsimd.partition_broadcast(recip_rep[:], recip[:])
                        nc.vector.tensor_mul(
                            out=ctx_sb[po : po + 64, p, :],
                            in0=cps[0:DK, :],
                            in1=recip_rep[:],
                        )
                    del ctx_pair[p]

            emit_front(0)
            pv_next = 0
            for gi in range(NG):
                while pv_next <= gi - max(LAG, 1):
                    emit_pv(pv_next)
                    pv_next += 1
                emit_mid(gi)
                if gi + 1 < NG:
                    emit_front(gi + 1)
                if LAG == 0 and pv_next <= gi:
                    emit_pv(pv_next)
                    pv_next += 1
            while pv_next < NG:
                emit_pv(pv_next)
                pv_next += 1

        # ---- output projection ----
        with tc.tile_pool(name="pout", bufs=2, space="PSUM") as pout:
            wo_sb = wpool.tile([128, DT, D], F16, tag="w")
            nc.sync.dma_start(out=wo_sb[:], in_=w["o"][:])
            for d in range(DT):
                ps = pout.tile([128, 512], F32, tag="po")
                for ka in range(DT):
                    nc.tensor.matmul(
                        ps[:],
                        wo_sb[:, ka, d * 128 : (d + 1) * 128],
                        ctx_sb[:, ka, :],
                        start=(ka == 0),
                        stop=(ka == DT - 1),
                    )
                o_sb = small.tile([128, SQ], F16, tag="osb")
                nc.scalar.activation(
                    out=o_sb[:],
                    in_=ps[:],
                    func=mybir.ActivationFunctionType.Identity,
                    bias=bias_sb["o"][:, d : d + 1],
                    scale=1.0,
                )
                nc.sync.dma_start(out=outT[d * 128 : (d + 1) * 128, :], in_=o_sb[:])


def _tile_dm(x, dtype=np.float16):
    """[D, N] -> [128, D//128, N] (partition-tiled over the first dim)."""
    n = x.shape[1]
    return np.ascontiguousarray(
        x.reshape(DT, 128, n).swapaxes(0, 1).astype(dtype)
    )


def _prep_inputs(q, k, v, mask, wq, bq, wk, bk, wv, bv, wo, bo):
    q = np.asarray(q, dtype=np.float32).reshape(S, D)
    k = np.asarray(k, dtype=np.float32).reshape(S, D)
    v = np.asarray(v, dtype=np.float32).reshape(S, D)
    mask = np.asarray(mask).reshape(S, S)

    def _chunk(x):
        # [128, 6, 4096] -> [128, 8, 6, 512] chunk-major contiguous
        return np.ascontiguousarray(
            x.reshape(128, DT, NCH, 512).transpose(0, 2, 1, 3)
        )

    kT_t = _chunk(_tile_dm(k.T))
    vT_t = _chunk(_tile_dm(v.T))  # [128, 8, 6, 512]; core c takes chunk c
    w_t = {
        "q": _tile_dm(np.asarray(wq, np.float32)),
        "k": _tile_dm(np.asarray(wk, np.float32)),
        "v": _tile_dm(np.asarray(wv, np.float32)),
        "o": _tile_dm(np.asarray(wo, np.float32)),
    }
    bcat = np.concatenate(
        [
            np.asarray(bb, np.float32).reshape(DT, 128).T
            for bb in (bq, bk, bv, bo)
        ]
        + [np.broadcast_to(np.asarray(bv, np.float32).reshape(1, H * DK), (128, H * DK))],
        axis=1,
    ).astype(np.float32)
    bcat = np.ascontiguousarray(bcat)

    in_maps = []
    for c in range(NCORES):
        qs, qe = c * SQ, (c + 1) * SQ
        m = {
            "qT": _tile_dm(q[qs:qe, :].T),
            "kT": kT_t,
            "vT": np.ascontiguousarray(vT_t[:, c]),
            "maskT": np.ascontiguousarray(
                (
                    mask[qs:qe, :].T.reshape(KT_TILES, 128, SQ)
                    .swapaxes(0, 1)
                    .reshape(128, KT_TILES, SQ // 4, 4)
                    * (1 << np.arange(4))
                ).sum(3)
            ).astype(ml_dtypes.float8_e4m3fn),
        }
        for x in "qkvo":
            m[f"w{x}"] = w_t[x]
        m["bcat"] = bcat
        in_maps.append(m)
    return in_maps


def kernel(**inputs) -> np.ndarray:
    if "nc" not in _CACHE:
        _CACHE["nc"] = build_kernel()
    nc = _CACHE["nc"]
    in_maps = _prep_inputs(**inputs)
    res = bass_utils.run_bass_kernel_spmd(nc, in_maps, core_ids=list(range(NCORES)))
    out = np.concatenate(
        [res.results[c]["outT"].T for c in range(NCORES)], axis=0
    ).astype(np.float32)
    return out.reshape(B, S, D)
